# revision 9
# baseline (speedup 1.0000x reference)
"""Multi-head causal self-attention (B=4, T=2048, C=384, H=6, Dh=64) on 8 trn2 cores.

Sharding: core c handles batch b = c//2 and head-group g = c%2 (3 heads each).
Each core computes its 3 heads' QKV projections, causal attention, and the
partial output projection (sum over its heads).  Host sums the two partials
per batch and adds the bias.

Device layout (per core):
  xt   [385, 2048]  x[b]^T with an appended row of ones (for denominator trick)
  wq   [384, 192]   Wq columns for the 3 heads
  wk   [384, 192]
  wv   [385, 260]   Wv columns rearranged to [V_h0|1|V_h1|1|V_h2|1|pad]-producing
                    layout (row 384 = ones-column indicator), padded to 260 cols
                    so the fp32r matmul moving dim is >= 256.
  wp   [192, 384]   Wp rows for the 3 heads
  msk  [128, 128]   additive causal mask for the diagonal 128x128 block

All matmuls run as float32r (full-rate fp32 on trn2 when moving dim >= 256).
Scores are computed transposed (S^T[k,q]) so softmax'd probabilities feed the
P@V matmul directly as the moving operand; the softmax denominator comes from
the appended ones column of V, and exp() folds the 1/sqrt(64) scale.
"""

import numpy as np
from contextlib import ExitStack

import concourse.bass as bass
from concourse import bacc
import concourse.tile as tile
from concourse import mybir
from concourse import bass_utils
from concourse._compat import with_exitstack

F32 = mybir.dt.float32
F32R = mybir.dt.float32r
AF = mybir.ActivationFunctionType

T = 2048
C = 384
NCORES = 8
NEG = -1.0e9

# set by test.py to collect a profile
PROFILE = False
LAST_RESULT = None


@with_exitstack
def _mha_body(ctx, tc, y, xt, wq, wk, wv, wp, msk):
    nc = tc.nc
    sb = ctx.enter_context(tc.tile_pool(name="sb", bufs=1))
    work = ctx.enter_context(tc.tile_pool(name="work", bufs=3))

    # ---------------- loads ----------------
    xt_sb = [sb.tile([128, T], F32R, tag=f"xt{c}", name=f"xt{c}") for c in range(3)]
    ones_sb = sb.tile([1, T], F32R, tag="xt_ones")
    for c in range(3):
        nc.sync.dma_start(xt_sb[c][:, :], xt[128 * c : 128 * (c + 1), :])
    nc.sync.dma_start(ones_sb[:, :], xt[384:385, :])

    wq_sb = [sb.tile([128, 192], F32R, tag=f"wq{c}", name=f"wq{c}") for c in range(3)]
    wk_sb = [sb.tile([128, 192], F32R, tag=f"wk{c}", name=f"wk{c}") for c in range(3)]
    for c in range(3):
        nc.sync.dma_start(wq_sb[c][:, :], wq[128 * c : 128 * (c + 1), :])
        nc.sync.dma_start(wk_sb[c][:, :], wk[128 * c : 128 * (c + 1), :])
    wv_sb = [sb.tile([128, 260], F32R, tag=f"wv{c}", name=f"wv{c}") for c in range(3)]
    wv_ones = sb.tile([1, 260], F32R, tag="wv_ones")
    for c in range(3):
        nc.sync.dma_start(wv_sb[c][:, :], wv[128 * c : 128 * (c + 1), :])
    nc.sync.dma_start(wv_ones[:, :], wv[384:385, :])
    wp_sb = [sb.tile([64, 384], F32R, tag=f"wp{h}", name=f"wp{h}") for h in range(3)]
    for h in range(3):
        nc.sync.dma_start(wp_sb[h][:, :], wp[64 * h : 64 * (h + 1), :])
    msk_sb = sb.tile([128, 128], F32, tag="msk")
    nc.sync.dma_start(msk_sb[:, :], msk[:, :])

    # persistent intermediates
    qt128 = sb.tile([128, T], F32R, tag="qt128")  # Q^T heads 0,1
    qt64 = sb.tile([64, T], F32R, tag="qt64")     # Q^T head 2
    kt128 = sb.tile([128, T], F32R, tag="kt128")
    kt64 = sb.tile([64, T], F32R, tag="kt64")
    ve = [sb.tile([128, 195], F32R, tag=f"ve{t}", name=f"ve{t}") for t in range(16)]  # [V|1] per k-chunk
    osc = [sb.tile([64, T], F32R, tag=f"osc{h}", name=f"osc{h}") for h in range(3)]    # normalized O^T

    # ---------------- phase 1: QKV projections ----------------
    with tc.tile_pool(name="ps_qkv", bufs=2, space="PSUM") as pq:
        for qt in range(4):
            for wsb, d128, d64 in ((wq_sb, qt128, qt64), (wk_sb, kt128, kt64)):
                ps = pq.tile([128, 512], F32, tag="qk")
                for c in range(3):
                    nc.tensor.matmul(
                        ps[:, :],
                        lhsT=wsb[c][:, 0:128],
                        rhs=xt_sb[c][:, 512 * qt : 512 * (qt + 1)],
                        start=(c == 0),
                        stop=(c == 2),
                    )
                nc.vector.tensor_copy(d128[:, 512 * qt : 512 * (qt + 1)], ps[:, :])
                ps2 = pq.tile([64, 512], F32, tag="qk64")
                for c in range(3):
                    nc.tensor.matmul(
                        ps2[:, :],
                        lhsT=wsb[c][:, 128:192],
                        rhs=xt_sb[c][:, 512 * qt : 512 * (qt + 1)],
                        start=(c == 0),
                        stop=(c == 2),
                    )
                nc.vector.tensor_copy(d64[:, 512 * qt : 512 * (qt + 1)], ps2[:, :])
        for t in range(16):
            ps = pq.tile([128, 260], F32, tag="v")
            for c in range(3):
                nc.tensor.matmul(
                    ps[:, :],
                    lhsT=xt_sb[c][:, 128 * t : 128 * (t + 1)],
                    rhs=wv_sb[c][:, :],
                    start=(c == 0),
                    stop=False,
                )
            nc.tensor.matmul(
                ps[:, :],
                lhsT=ones_sb[:, 128 * t : 128 * (t + 1)],
                rhs=wv_ones[:, :],
                start=False,
                stop=True,
            )
            nc.vector.tensor_copy(ve[t][:, :], ps[:, 0:195])

    # ---------------- phase 2: causal attention ----------------
    def QT(h):
        return qt128[64 * h : 64 * (h + 1), :] if h < 2 else qt64[:, :]

    def KT(h):
        return kt128[64 * h : 64 * (h + 1), :] if h < 2 else kt64[:, :]

    with tc.tile_pool(name="ps_at", bufs=1, space="PSUM") as pa:
        for h in range(3):
            qth, kth = QT(h), KT(h)
            for p in range(2):  # q-pair: q in [1024p, 1024p+1024)
                q0 = 1024 * p
                ot = [pa.tile([65, 512], F32, tag=f"ot{half}", name=f"ot{half}_{h}_{p}") for half in range(2)]
                for j in range(8 * p + 8):  # k-chunks of 128
                    m = j - 8 * p  # >=0: diagonal chunk at col 128m of this pair
                    sp = pa.tile([128, 1024], F32, tag="sp", bufs=2)
                    for half in range(2):
                        qoff = 512 * half
                        lo = max(0, 128 * m - qoff) if m >= 0 else 0
                        if lo >= 512:
                            continue
                        nc.tensor.matmul(
                            sp[:, qoff + lo : qoff + 512],
                            lhsT=kth[:, 128 * j : 128 * (j + 1)],
                            rhs=qth[:, q0 + qoff + lo : q0 + qoff + 512],
                            start=True,
                            stop=True,
                        )
                    if m >= 0:
                        col = 128 * m
                        nc.vector.tensor_add(
                            sp[:, col : col + 128], sp[:, col : col + 128], msk_sb[:, :]
                        )
                    lo_e = 128 * m if m >= 0 else 0
                    pt = work.tile([128, 1024], F32R, tag="pt")
                    nc.scalar.activation(
                        pt[:, lo_e:1024], sp[:, lo_e:1024], AF.Exp, scale=0.125
                    )
                    for half in range(2):
                        qoff = 512 * half
                        lo = max(0, 128 * m - qoff) if m >= 0 else 0
                        if lo >= 512:
                            continue
                        nc.tensor.matmul(
                            ot[half][:, lo:512],
                            lhsT=ve[j][:, 65 * h : 65 * (h + 1)],
                            rhs=pt[:, qoff + lo : qoff + 512],
                            start=(j == 0),
                            stop=(j == 8 * p + 4 * half + 3),
                        )
                for half in range(2):
                    r = work.tile([1, 512], F32R, tag="r")
                    with nc.allow_low_precision(reason="fp32r rhs for bcast matmul"):
                        nc.vector.reciprocal(r[:, :], ot[half][64:65, :])
                    bcp = pa.tile([64, 512], F32, tag="bcp", bufs=2)
                    nc.tensor.matmul(
                        bcp[:, :],
                        lhsT=ones_sb[:, 0:64],
                        rhs=r[:, :],
                        start=True,
                        stop=True,
                    )
                    bc = work.tile([64, 512], F32, tag="bc")
                    nc.vector.tensor_copy(bc[:, :], bcp[:, :])
                    nc.vector.tensor_mul(
                        osc[h][:, q0 + 512 * half : q0 + 512 * (half + 1)],
                        ot[half][0:64, :],
                        bc[:, :],
                    )

    # ---------------- phase 3: output projection (partial over 3 heads) ----
    with tc.tile_pool(name="ps_pr", bufs=2, space="PSUM") as pp:
        for t in range(16):
            yp = pp.tile([128, 384], F32, tag="y")
            for h in range(3):
                nc.tensor.matmul(
                    yp[:, :],
                    lhsT=osc[h][:, 128 * t : 128 * (t + 1)],
                    rhs=wp_sb[h][:, :],
                    start=(h == 0),
                    stop=(h == 2),
                )
            ys = work.tile([128, 384], F32, tag="ys")
            nc.vector.tensor_copy(ys[:, :], yp[:, :])
            nc.sync.dma_start(y[128 * t : 128 * (t + 1), :], ys[:, :])


def build_program():
    nc = bacc.Bacc("TRN2", debug=False, num_devices=NCORES)
    xt = nc.dram_tensor("xt", [C + 1, T], F32R, kind="ExternalInput").ap()
    wq = nc.dram_tensor("wq", [C, 192], F32R, kind="ExternalInput").ap()
    wk = nc.dram_tensor("wk", [C, 192], F32R, kind="ExternalInput").ap()
    wv = nc.dram_tensor("wv", [C + 1, 260], F32R, kind="ExternalInput").ap()
    wp = nc.dram_tensor("wp", [192, 384], F32R, kind="ExternalInput").ap()
    msk = nc.dram_tensor("msk", [128, 128], F32, kind="ExternalInput").ap()
    y = nc.dram_tensor("y", [T, C], F32, kind="ExternalOutput").ap()
    with tile.TileContext(nc) as tc:
        _mha_body(tc, y, xt, wq, wk, wv, wp, msk)
    nc.compile()
    return nc


def make_in_maps(x, Wq, Wk, Wv, Wp):
    """Build the 8 per-core input dicts from the full tensors."""
    msk = np.where(
        np.arange(128)[:, None] <= np.arange(128)[None, :], 0.0, NEG
    ).astype(np.float32)
    in_maps = []
    for core in range(NCORES):
        b, g = core // 2, core % 2
        xt = np.empty((C + 1, T), np.float32)
        xt[:C] = x[b].T
        xt[C] = 1.0
        wv_arr = np.zeros((C + 1, 260), np.float32)
        for hh in range(3):
            col = 192 * g + 64 * hh
            wv_arr[0:C, 65 * hh : 65 * hh + 64] = Wv[:, col : col + 64]
            wv_arr[C, 65 * hh + 64] = 1.0
        in_maps.append(
            {
                "xt": xt,
                "wq": np.ascontiguousarray(Wq[:, 192 * g : 192 * (g + 1)]),
                "wk": np.ascontiguousarray(Wk[:, 192 * g : 192 * (g + 1)]),
                "wv": wv_arr,
                "wp": np.ascontiguousarray(Wp[192 * g : 192 * (g + 1), :]),
                "msk": msk,
            }
        )
    return in_maps


_program_cache = {}


def kernel(x, Wq, Wk, Wv, Wp, bp):
    global LAST_RESULT
    x = np.asarray(x, np.float32)
    Wq = np.asarray(Wq, np.float32)
    Wk = np.asarray(Wk, np.float32)
    Wv = np.asarray(Wv, np.float32)
    Wp = np.asarray(Wp, np.float32)
    bp = np.asarray(bp, np.float32)
    B = x.shape[0]

    if "nc" not in _program_cache:
        _program_cache["nc"] = build_program()
    nc = _program_cache["nc"]

    in_maps = make_in_maps(x, Wq, Wk, Wv, Wp)
    res = bass_utils.run_bass_kernel_spmd(
        nc, in_maps, core_ids=list(range(NCORES)), trace=PROFILE
    )
    LAST_RESULT = res
    outs = [np.asarray(r["y"], np.float32) for r in res.results]
    y = np.stack([outs[2 * b] + outs[2 * b + 1] + bp for b in range(B)], axis=0)
    return y.astype(np.float32)
